# revision 2
# baseline (speedup 1.0000x reference)
"""Trainium2 Bass kernel for a GPT-style transformer block.

B=4, T=2048, C=1024, H=16 heads (D=64), FF=4096.
Sharding: 8 NeuronCores, core c = 2*b + h handles batch b, token half h
(queries/output tokens [h*1024, (h+1)*1024)); K/V are computed on-core over
the full sequence. One uniform SPMD program; per-core causality enters only
through data (host-rotated x and DMA'd multiplicative masks).

All matmuls run in float32r (~tf32 precision at near-bf16 speed).
Activations stay channel-major ([channel, token]) end to end: LN statistics
are taken over the partition dim with ones-matmuls, softmax runs without max
subtraction (scores are bounded), and V is augmented with a ones column so
the softmax denominator falls out of the AV matmul itself.
"""
import sys

sys.path.insert(0, "/opt/trn_rl_repo")

import numpy as np
import ml_dtypes
from contextlib import ExitStack

import concourse.bass as bass
import concourse.tile as tile
from concourse import bacc, mybir

F32 = mybir.dt.float32
F32R = mybir.dt.float32r
BF16 = mybir.dt.bfloat16
AF = mybir.ActivationFunctionType
OP = mybir.AluOpType

B, T, C, H, D = 4, 2048, 1024, 16, 64
FF = 4 * C
TOK = T // 2          # tokens owned per core
NCB = C // 128        # 8 channel blocks
NG = 2                # head groups
GH = H // NG          # 8 heads per group
NPAIR = GH // 2       # 4 head pairs per group
NSB = T // 128        # 16 s-blocks
NV = 12 + 16          # mask visits per head pair (qc=0: 12, qc=1: 16)

_CACHE = {}


def _build():
    nc = bacc.Bacc("TRN2", target_bir_lowering=False, debug=False, num_devices=8)

    xT_d = nc.dram_tensor("xT", [C, T], F32, kind="ExternalInput").ap()
    wqkv_d = nc.dram_tensor("wqkvT", [C, 3 * C], F32, kind="ExternalInput").ap()
    wo_d = nc.dram_tensor("woT", [C, C], F32, kind="ExternalInput").ap()
    wfc_d = nc.dram_tensor("wfcT", [C, FF], F32, kind="ExternalInput").ap()
    wproj_d = nc.dram_tensor("wprojT", [FF, C], F32, kind="ExternalInput").ap()
    mask_d = nc.dram_tensor("masks", [NV, 128, 512], BF16, kind="ExternalInput").ap()
    out_d = nc.dram_tensor("outT", [C, TOK], F32, kind="ExternalOutput").ap()
    y_dram = nc.dram_tensor("ydram", [C, TOK], F32).ap()   # internal: normalized attn y

    with tile.TileContext(nc) as tc, ExitStack() as top:
        persist = top.enter_context(tc.tile_pool(name="persist", bufs=1))
        ones_f = persist.tile([128, 128], F32)
        nc.vector.memset(ones_f[:], 1.0)
        ones_r = persist.tile([128, 128], F32R)
        nc.vector.tensor_copy(ones_r[:], ones_f[:])
        eps_t = persist.tile([128, 1], F32)
        nc.vector.memset(eps_t[:], 1e-5)

        pa = top.enter_context(tc.tile_pool(name="pa", bufs=1))  # LN1 stat vectors
        mean_r = pa.tile([1, T], F32)
        rstd_r = pa.tile([1, T], F32)

        # ============ Phase A: LN1 statistics over channels ============
        with tc.tile_pool(name="pax", bufs=3) as pax, \
             tc.tile_pool(name="psA", bufs=2, space="PSUM") as psA:
            for tck in range(T // 512):
                sl = slice(tck * 512, (tck + 1) * 512)
                sum_ps = psA.tile([1, 512], F32, tag="sum")
                sq_ps = psA.tile([1, 512], F32, tag="sq")
                for cb in range(NCB):
                    xf = pax.tile([128, 512], F32, tag="xf")
                    nc.sync.dma_start(xf[:], xT_d[cb * 128:(cb + 1) * 128, sl])
                    xr = pax.tile([128, 512], F32R, tag="xr")
                    nc.vector.tensor_copy(xr[:], xf[:])
                    x2 = pax.tile([128, 512], F32R, tag="x2")
                    nc.vector.tensor_tensor(x2[:], xf[:], xf[:], OP.mult)
                    nc.tensor.matmul(sum_ps[:], ones_r[:, 0:1], xr[:],
                                     start=(cb == 0), stop=(cb == NCB - 1))
                    nc.tensor.matmul(sq_ps[:], ones_r[:, 0:1], x2[:],
                                     start=(cb == 0), stop=(cb == NCB - 1))
                nc.scalar.mul(mean_r[:, sl], sum_ps[:], 1.0 / C)
                nc.scalar.mul(rstd_r[:, sl], sq_ps[:], 1.0 / C)
            msq = pax.tile([1, T], F32, tag="msq")
            nc.vector.tensor_mul(msq[:], mean_r[:], mean_r[:])
            nc.vector.tensor_sub(rstd_r[:], rstd_r[:], msq[:])
            nc.scalar.activation(rstd_r[:], rstd_r[:], AF.Sqrt, bias=eps_t[0:1, :])
            nc.vector.reciprocal(rstd_r[:], rstd_r[:])

        # ============ Phases B+C per head group ============
        with tc.tile_pool(name="pb", bufs=1) as pb:
            masks = pb.tile([128, NV, 512], BF16)
            for v in range(NV):
                nc.sync.dma_start(masks[:, v, :], mask_d[v])

            for g in range(NG):
                # ---- Phase B: QKV for this group ----
                kT_g = pb.tile([128, NPAIR, T], F32R, tag="kT")
                qT_g = pb.tile([128, NPAIR, TOK], F32R, tag="qT")
                v_aug = pb.tile([128, NSB, GH, 65], F32R, tag="vaug")
                nc.vector.tensor_copy(v_aug[:, :, :, 64:65], ones_r[:, 0:NSB * GH])

                with tc.tile_pool(name="pbt", bufs=1) as pbt, \
                     tc.tile_pool(name="pbx", bufs=2) as pbx, \
                     tc.tile_pool(name="pbw", bufs=3) as pbw, \
                     tc.tile_pool(name="pbv", bufs=1) as pbv, \
                     tc.tile_pool(name="psB", bufs=2, space="PSUM") as psB:
                    for tck in range(2):  # t-chunks of 1024 over the full sequence
                        tsl = slice(tck * 1024, (tck + 1) * 1024)
                        mb = pbt.tile([128, 1024], F32, tag="mb")
                        rb = pbt.tile([128, 1024], F32, tag="rb")
                        nc.gpsimd.partition_broadcast(mb[:], mean_r[:, tsl])
                        nc.gpsimd.partition_broadcast(rb[:], rstd_r[:, tsl])
                        lnr = pbt.tile([128, NCB, 1024], F32R, tag="lnr")
                        for cb in range(NCB):
                            xf = pbx.tile([128, 1024], F32, tag="xbf")
                            nc.sync.dma_start(xf[:], xT_d[cb * 128:(cb + 1) * 128, tsl])
                            nc.vector.tensor_sub(xf[:], xf[:], mb[:])
                            nc.vector.tensor_tensor(lnr[:, cb, :], xf[:], rb[:], OP.mult)
                        # k for every chunk; q only for the own-token chunk (tck==1)
                        ocb_list = (list(range(NPAIR, 2 * NPAIR)) if tck == 0
                                    else list(range(2 * NPAIR)))
                        for ocb in ocb_list:
                            is_q = ocb < NPAIR
                            pblk = ocb % NPAIR
                            col0 = (0 if is_q else C) + g * 512 + pblk * 128
                            acc = psB.tile([128, 1024], F32, tag="qk")
                            for cb in range(NCB):
                                wf = pbw.tile([128, 128], F32, tag="wf")
                                nc.sync.dma_start(wf[:], wqkv_d[cb * 128:(cb + 1) * 128,
                                                               col0:col0 + 128])
                                wr = pbw.tile([128, 128], F32R, tag="wr")
                                nc.vector.tensor_copy(wr[:], wf[:])
                                for n2 in range(2):
                                    nc.tensor.matmul(acc[:, n2 * 512:(n2 + 1) * 512],
                                                     wr[:], lnr[:, cb, n2 * 512:(n2 + 1) * 512],
                                                     start=(cb == 0), stop=(cb == NCB - 1))
                            if is_q:
                                nc.vector.tensor_copy(qT_g[:, pblk, :], acc[:])
                            else:
                                nc.vector.tensor_copy(kT_g[:, pblk, tsl], acc[:])
                        # v for this chunk: token-major augmented
                        vw = []
                        for cb in range(NCB):
                            vwf = pbw.tile([128, 512], F32, tag="vwf")
                            nc.sync.dma_start(vwf[:], wqkv_d[cb * 128:(cb + 1) * 128,
                                                            2 * C + g * 512: 2 * C + (g + 1) * 512])
                            vwr = pbv.tile([128, 512], F32R, tag=f"vwr{cb}")
                            nc.vector.tensor_copy(vwr[:], vwf[:])
                            vw.append(vwr)
                        for sb_l in range(8):
                            sblk = tck * 8 + sb_l
                            vps = psB.tile([128, 512], F32, tag="vps")
                            for cb in range(NCB):
                                nc.tensor.matmul(vps[:], lnr[:, cb, sb_l * 128:(sb_l + 1) * 128],
                                                 vw[cb][:], start=(cb == 0), stop=(cb == NCB - 1))
                            nc.vector.tensor_copy(v_aug[:, sblk, :, 0:64], vps[:])

                # ---- Phase C: attention for this group ----
                with tc.tile_pool(name="pct", bufs=2) as pct, \
                     tc.tile_pool(name="psS", bufs=2, space="PSUM") as psS, \
                     tc.tile_pool(name="psY", bufs=2, space="PSUM") as psY:
                    for qc in range(2):
                        qsl = slice(qc * 512, (qc + 1) * 512)
                        trip = 12 + 4 * qc
                        vbase = 0 if qc == 0 else 12
                        for pair in range(NPAIR):
                            y0 = psY.tile([65, 512], F32, tag="y0")
                            y1 = psY.tile([65, 512], F32, tag="y1")
                            for j in range(trip):
                                st = (j == 0)
                                sp = (j == trip - 1)
                                s0 = psS.tile([128, 512], F32, tag="s0")
                                s1 = psS.tile([128, 512], F32, tag="s1")
                                nc.tensor.matmul(s0[:], kT_g[0:64, pair, j * 128:(j + 1) * 128],
                                                 qT_g[0:64, pair, qsl], start=True, stop=True,
                                                 tile_position=(0, 0))
                                nc.tensor.matmul(s1[:], kT_g[64:128, pair, j * 128:(j + 1) * 128],
                                                 qT_g[64:128, pair, qsl], start=True, stop=True,
                                                 tile_position=(64, 0))
                                p0 = pct.tile([128, 512], F32, tag="p0")
                                p1 = pct.tile([128, 512], F32, tag="p1")
                                nc.scalar.activation(p0[:], s0[:], AF.Exp, scale=0.125)
                                nc.scalar.activation(p1[:], s1[:], AF.Exp, scale=0.125)
                                p0r = pct.tile([128, 512], F32R, tag="p0r")
                                p1r = pct.tile([128, 512], F32R, tag="p1r")
                                nc.vector.tensor_tensor(p0r[:], p0[:], masks[:, vbase + j, :], OP.mult)
                                nc.vector.tensor_tensor(p1r[:], p1[:], masks[:, vbase + j, :], OP.mult)
                                nc.tensor.matmul(y0[:], v_aug[:, j, 2 * pair, :], p0r[:],
                                                 start=st, stop=sp)
                                nc.tensor.matmul(y1[:], v_aug[:, j, 2 * pair + 1, :], p1r[:],
                                                 start=st, stop=sp)
                            rec0 = pct.tile([1, 512], F32, tag="rec0")
                            rec1 = pct.tile([1, 512], F32, tag="rec1")
                            nc.vector.reciprocal(rec0[:], y0[64:65, :])
                            nc.vector.reciprocal(rec1[:], y1[64:65, :])
                            rb0 = pct.tile([64, 512], F32, tag="rb0")
                            rb1 = pct.tile([64, 512], F32, tag="rb1")
                            nc.gpsimd.partition_broadcast(rb0[:], rec0[:], channels=64)
                            nc.gpsimd.partition_broadcast(rb1[:], rec1[:], channels=64)
                            blk = g * NPAIR + pair
                            yn = pct.tile([128, 512], F32R, tag="yn")
                            nc.vector.tensor_tensor(yn[0:64, :], y0[0:64, :], rb0[:], OP.mult)
                            nc.vector.tensor_tensor(yn[64:128, :], y1[0:64, :], rb1[:], OP.mult)
                            nc.sync.dma_start(y_dram[blk * 128:(blk + 1) * 128, qsl],
                                              yn[:].bitcast(F32))

        # ============ Phase D: out-projection + residual + LN2 ============
        with tc.tile_pool(name="pd", bufs=1) as pd:
            x1 = pd.tile([128, NCB, TOK], F32)
            with tc.tile_pool(name="pdw", bufs=3) as pdw, \
                 tc.tile_pool(name="pdt", bufs=2) as pdt, \
                 tc.tile_pool(name="psD", bufs=2, space="PSUM") as psD:
                for ocb in range(NCB):
                    acc = psD.tile([128, 1024], F32, tag="proj")
                    for cblk in range(NCB):
                        wf = pdw.tile([128, 128], F32, tag="wf")
                        nc.sync.dma_start(wf[:], wo_d[cblk * 128:(cblk + 1) * 128,
                                                      ocb * 128:(ocb + 1) * 128])
                        wr = pdw.tile([128, 128], F32R, tag="wr")
                        nc.vector.tensor_copy(wr[:], wf[:])
                        for n2 in range(2):
                            yt = pdt.tile([128, 512], F32R, tag="yt")
                            nc.sync.dma_start(yt[:].bitcast(F32),
                                              y_dram[cblk * 128:(cblk + 1) * 128,
                                                     n2 * 512:(n2 + 1) * 512])
                            nc.tensor.matmul(acc[:, n2 * 512:(n2 + 1) * 512], wr[:], yt[:],
                                             start=(cblk == 0), stop=(cblk == NCB - 1))
                    xf = pdt.tile([128, TOK], F32, tag="xres")
                    # own tokens are chunk 1 after host rotation
                    nc.sync.dma_start(xf[:], xT_d[ocb * 128:(ocb + 1) * 128, TOK:T])
                    nc.vector.tensor_add(x1[:, ocb, :], acc[:], xf[:])

            # LN2 stats + normalize
            ln2r = pd.tile([128, NCB, TOK], F32R)
            with tc.tile_pool(name="pet", bufs=3) as pet, \
                 tc.tile_pool(name="psE", bufs=2, space="PSUM") as psE:
                m2 = pd.tile([1, TOK], F32)
                v2 = pd.tile([1, TOK], F32)
                for tcks in range(TOK // 512):
                    sl = slice(tcks * 512, (tcks + 1) * 512)
                    sum_ps = psE.tile([1, 512], F32, tag="sum")
                    sq_ps = psE.tile([1, 512], F32, tag="sq")
                    for cb in range(NCB):
                        xr = pet.tile([128, 512], F32R, tag="xr")
                        nc.vector.tensor_copy(xr[:], x1[:, cb, sl])
                        x2 = pet.tile([128, 512], F32R, tag="x2")
                        nc.vector.tensor_tensor(x2[:], x1[:, cb, sl], x1[:, cb, sl], OP.mult)
                        nc.tensor.matmul(sum_ps[:], ones_r[:, 0:1], xr[:],
                                         start=(cb == 0), stop=(cb == NCB - 1))
                        nc.tensor.matmul(sq_ps[:], ones_r[:, 0:1], x2[:],
                                         start=(cb == 0), stop=(cb == NCB - 1))
                    nc.scalar.mul(m2[:, sl], sum_ps[:], 1.0 / C)
                    nc.scalar.mul(v2[:, sl], sq_ps[:], 1.0 / C)
                ms2 = pd.tile([1, TOK], F32)
                nc.vector.tensor_mul(ms2[:], m2[:], m2[:])
                nc.vector.tensor_sub(v2[:], v2[:], ms2[:])
                nc.scalar.activation(v2[:], v2[:], AF.Sqrt, bias=eps_t[0:1, :])
                nc.vector.reciprocal(v2[:], v2[:])
                mean2_b = pd.tile([128, TOK], F32)
                rstd2_b = pd.tile([128, TOK], F32)
                nc.gpsimd.partition_broadcast(mean2_b[:], m2[:])
                nc.gpsimd.partition_broadcast(rstd2_b[:], v2[:])
                for cb in range(NCB):
                    xc = pet.tile([128, TOK], F32, tag="xc2")
                    nc.vector.tensor_sub(xc[:], x1[:, cb, :], mean2_b[:])
                    nc.vector.tensor_tensor(ln2r[:, cb, :], xc[:], rstd2_b[:], OP.mult)

            # ============ Phase E: MLP ============
            with tc.tile_pool(name="ph", bufs=1) as ph, \
                 tc.tile_pool(name="pew", bufs=3) as pew, \
                 tc.tile_pool(name="psF", bufs=2, space="PSUM") as psF:
                for tcks in range(TOK // 512):
                    sl = slice(tcks * 512, (tcks + 1) * 512)
                    h_r = ph.tile([128, FF // 128, 512], F32R, tag="h")
                    for fb in range(FF // 128):
                        fc = psF.tile([128, 512], F32, tag="fc")
                        for cb in range(NCB):
                            wf = pew.tile([128, 128], F32, tag="fwf")
                            nc.sync.dma_start(wf[:], wfc_d[cb * 128:(cb + 1) * 128,
                                                           fb * 128:(fb + 1) * 128])
                            wr = pew.tile([128, 128], F32R, tag="fwr")
                            nc.vector.tensor_copy(wr[:], wf[:])
                            nc.tensor.matmul(fc[:], wr[:], ln2r[:, cb, sl],
                                             start=(cb == 0), stop=(cb == NCB - 1))
                        nc.scalar.activation(h_r[:, fb, :], fc[:], AF.Gelu)
                    for ocb in range(NCB):
                        acc = psF.tile([128, 512], F32, tag="pacc")
                        for fb in range(FF // 128):
                            wf = pew.tile([128, 128], F32, tag="pwf")
                            nc.sync.dma_start(wf[:], wproj_d[fb * 128:(fb + 1) * 128,
                                                             ocb * 128:(ocb + 1) * 128])
                            wr = pew.tile([128, 128], F32R, tag="pwr")
                            nc.vector.tensor_copy(wr[:], wf[:])
                            nc.tensor.matmul(acc[:], wr[:], h_r[:, fb, :],
                                             start=(fb == 0), stop=(fb == FF // 128 - 1))
                        of = pew.tile([128, 512], F32, tag="of")
                        nc.vector.tensor_add(of[:], acc[:], x1[:, ocb, sl])
                        nc.sync.dma_start(out_d[ocb * 128:(ocb + 1) * 128, sl], of[:])

    nc.compile()
    return nc


def _prep(x, g1, w_qkv, w_o, g2, w_fc, w_proj):
    """Build the 8 per-core input maps (all host-side, fp32)."""
    x = np.asarray(x, np.float32)
    g1 = np.asarray(g1, np.float32)
    g2 = np.asarray(g2, np.float32)
    wqkvT = np.ascontiguousarray((np.asarray(w_qkv, np.float32) * g1[None, :]).T)
    woT = np.ascontiguousarray(np.asarray(w_o, np.float32).T)
    wfcT = np.ascontiguousarray((np.asarray(w_fc, np.float32) * g2[None, :]).T)
    wprojT = np.ascontiguousarray(np.asarray(w_proj, np.float32).T)

    in_maps = []
    for c in range(8):
        b, h = c // 2, c % 2
        # Rotate the sequence so the core's own tokens are always chunk 1
        # ([TOK:T]) of xT: h=0 swaps halves, h=1 keeps order.
        xb = x[b]
        if h == 0:
            xb = np.concatenate([xb[TOK:], xb[:TOK]], axis=0)
        xT = np.ascontiguousarray(xb.T)
        # Causal masks in ROTATED key coordinates.
        rot = (np.arange(T) + (TOK if h == 0 else 0)) % T
        gq = rot[TOK:]           # global positions of own (query) tokens
        gk = rot                 # global positions of keys in rotated order
        masks = np.zeros((NV, 128, 512), np.float32)
        idx = 0
        for qc in range(2):
            qpos = gq[qc * 512:(qc + 1) * 512]
            for j in range(12 + 4 * qc):
                kpos = gk[j * 128:(j + 1) * 128]
                masks[idx] = (kpos[:, None] <= qpos[None, :])
                idx += 1
        assert idx == NV
        in_maps.append({
            "xT": xT,
            "wqkvT": wqkvT, "woT": woT, "wfcT": wfcT, "wprojT": wprojT,
            "masks": masks.astype(ml_dtypes.bfloat16),
        })
    return in_maps


def kernel(x, g1, w_qkv, w_o, g2, w_fc, w_proj, _trace=False, **_tk):
    from concourse.bass_utils import run_bass_kernel_spmd
    if "nc" not in _CACHE:
        _CACHE["nc"] = _build()
    nc = _CACHE["nc"]
    in_maps = _prep(x, g1, w_qkv, w_o, g2, w_fc, w_proj)
    res = run_bass_kernel_spmd(nc, in_maps, core_ids=list(range(8)),
                               trace=_trace, **_tk)
    _CACHE["last"] = res
    out = np.empty((B, T, C), np.float32)
    for c in range(8):
        b, h = c // 2, c % 2
        out[b, h * TOK:(h + 1) * TOK, :] = res.results[c]["outT"].T
    return out


# revision 4
# speedup vs baseline: 1.1853x; 1.1853x over previous
"""Trainium2 Bass kernel for a GPT-style transformer block.

B=4, T=2048, C=1024, H=16 heads (D=64), FF=4096.
Sharding: 8 NeuronCores, core c = 2*b + h handles batch b, token half h
(queries/output tokens [h*1024, (h+1)*1024)); K/V are computed on-core over
the full sequence. One uniform SPMD program; per-core causality enters only
through data (host-rotated x and DMA'd multiplicative masks).

All matmuls run in float32r (fp32 rounded to 11 mantissa bits, RNE — ~4x
the precision of bf16 at near-bf16 PE throughput). Weights are pre-rounded
and tile-packed on the host so each weight tile is one contiguous 64 KiB
DMA burst straight into an F32R SBUF tile. Activations stay channel-major
([channel, token]) end to end: LN statistics are taken over the partition
dim with ones-matmuls, softmax runs without max subtraction (scores are
bounded), and V is augmented with a ones column so the softmax denominator
falls out of the AV matmul itself.
"""
import sys

sys.path.insert(0, "/opt/trn_rl_repo")

import numpy as np
import ml_dtypes
from contextlib import ExitStack

import concourse.bass as bass
import concourse.tile as tile
from concourse import bacc, mybir

F32 = mybir.dt.float32
F32R = mybir.dt.float32r
BF16 = mybir.dt.bfloat16
AF = mybir.ActivationFunctionType
OP = mybir.AluOpType

B, T, C, H, D = 4, 2048, 1024, 16, 64
FF = 4 * C
TOK = T // 2          # tokens owned per core
NCB = C // 128        # 8 channel blocks
NFB = FF // 128       # 32 ff blocks
NG = 2                # head groups
GH = H // NG          # 8 heads per group
NPAIR = GH // 2       # 4 head pairs per group
NSB = T // 128        # 16 s-blocks
NV = 12 + 16          # mask visits per head pair (qc=0: 12, qc=1: 16)

_CACHE = {}


def _build():
    nc = bacc.Bacc("TRN2", target_bir_lowering=False, debug=False, num_devices=8)

    xT_d = nc.dram_tensor("xT", [C, T], F32, kind="ExternalInput").ap()
    wqk_d = nc.dram_tensor("wqk", [NG, 2 * NPAIR, NCB, 128, 128], F32R,
                           kind="ExternalInput").ap()
    wv_d = nc.dram_tensor("wv", [NG, NCB, 128, 512], F32R, kind="ExternalInput").ap()
    wo_d = nc.dram_tensor("wo", [NCB, NCB, 128, 128], F32R, kind="ExternalInput").ap()
    wfc_d = nc.dram_tensor("wfc", [NFB, NCB, 128, 128], F32R, kind="ExternalInput").ap()
    wproj_d = nc.dram_tensor("wproj", [NCB, NFB, 128, 128], F32R,
                             kind="ExternalInput").ap()
    mask_d = nc.dram_tensor("masks", [NV, 128, 512], BF16, kind="ExternalInput").ap()
    out_d = nc.dram_tensor("outT", [C, TOK], F32, kind="ExternalOutput").ap()
    y_dram = nc.dram_tensor("ydram", [C, TOK], F32).ap()   # internal: normalized attn y

    with tile.TileContext(nc) as tc, ExitStack() as top:
        persist = top.enter_context(tc.tile_pool(name="persist", bufs=1))
        ones_f = persist.tile([128, 128], F32)
        nc.vector.memset(ones_f[:], 1.0)
        ones_r = persist.tile([128, 128], F32R)
        nc.vector.tensor_copy(ones_r[:], ones_f[:])
        eps_t = persist.tile([128, 1], F32)
        nc.vector.memset(eps_t[:], 1e-5)

        pa = top.enter_context(tc.tile_pool(name="pa", bufs=1))  # LN1 stat vectors
        mean_r = pa.tile([1, T], F32)
        rstd_r = pa.tile([1, T], F32)

        # ============ Phase A: LN1 statistics over channels ============
        with tc.tile_pool(name="pax", bufs=3) as pax, \
             tc.tile_pool(name="psA", bufs=2, space="PSUM") as psA:
            for tck in range(T // 512):
                sl = slice(tck * 512, (tck + 1) * 512)
                sum_ps = psA.tile([1, 512], F32, tag="sum")
                sq_ps = psA.tile([1, 512], F32, tag="sq")
                for cb in range(NCB):
                    xf = pax.tile([128, 512], F32, tag="xf")
                    nc.sync.dma_start(xf[:], xT_d[cb * 128:(cb + 1) * 128, sl])
                    xr = pax.tile([128, 512], F32R, tag="xr")
                    nc.vector.tensor_copy(xr[:], xf[:])
                    x2 = pax.tile([128, 512], F32R, tag="x2")
                    nc.vector.tensor_tensor(x2[:], xf[:], xf[:], OP.mult)
                    nc.tensor.matmul(sum_ps[:], ones_r[:, 0:1], xr[:],
                                     start=(cb == 0), stop=(cb == NCB - 1))
                    nc.tensor.matmul(sq_ps[:], ones_r[:, 0:1], x2[:],
                                     start=(cb == 0), stop=(cb == NCB - 1))
                nc.scalar.mul(mean_r[:, sl], sum_ps[:], 1.0 / C)
                nc.scalar.mul(rstd_r[:, sl], sq_ps[:], 1.0 / C)
            msq = pax.tile([1, T], F32, tag="msq")
            nc.vector.tensor_mul(msq[:], mean_r[:], mean_r[:])
            nc.vector.tensor_sub(rstd_r[:], rstd_r[:], msq[:])
            nc.scalar.activation(rstd_r[:], rstd_r[:], AF.Sqrt, bias=eps_t[0:1, :])
            nc.vector.reciprocal(rstd_r[:], rstd_r[:])

        # ============ Phases B+C per head group ============
        with tc.tile_pool(name="pb", bufs=1) as pb:
            masks = pb.tile([128, NV, 512], BF16)
            for v in range(NV):
                nc.sync.dma_start(masks[:, v, :], mask_d[v])

            for g in range(NG):
                # ---- Phase B: QKV for this group ----
                kT_g = pb.tile([128, NPAIR, T], F32R, tag="kT")
                qT_g = pb.tile([128, NPAIR, TOK], F32R, tag="qT")
                v_aug = pb.tile([128, NSB, GH, 65], F32R, tag="vaug")
                nc.vector.tensor_copy(v_aug[:, :, :, 64:65], ones_r[:, 0:NSB * GH])

                with tc.tile_pool(name="pbt", bufs=1) as pbt, \
                     tc.tile_pool(name="pbx", bufs=2) as pbx, \
                     tc.tile_pool(name="pbw", bufs=6) as pbw, \
                     tc.tile_pool(name="pbv", bufs=1) as pbv, \
                     tc.tile_pool(name="psB", bufs=2, space="PSUM") as psB:
                    # v weights: once per group
                    vw = []
                    for cb in range(NCB):
                        vwr = pbv.tile([128, 512], F32R, tag=f"vwr{cb}")
                        nc.sync.dma_start(vwr[:], wv_d[g, cb])
                        vw.append(vwr)
                    for tck in range(2):  # t-chunks of 1024 over the full sequence
                        tsl = slice(tck * 1024, (tck + 1) * 1024)
                        mb = pbt.tile([128, 1024], F32, tag="mb")
                        rb = pbt.tile([128, 1024], F32, tag="rb")
                        nc.gpsimd.partition_broadcast(mb[:], mean_r[:, tsl])
                        nc.gpsimd.partition_broadcast(rb[:], rstd_r[:, tsl])
                        lnr = pbt.tile([128, NCB, 1024], F32R, tag="lnr")
                        for cb in range(NCB):
                            xf = pbx.tile([128, 1024], F32, tag="xbf")
                            nc.sync.dma_start(xf[:], xT_d[cb * 128:(cb + 1) * 128, tsl])
                            nc.vector.tensor_sub(xf[:], xf[:], mb[:])
                            nc.vector.tensor_tensor(lnr[:, cb, :], xf[:], rb[:], OP.mult)
                        # k for every chunk; q only for the own-token chunk (tck==1)
                        ocb_list = (list(range(NPAIR, 2 * NPAIR)) if tck == 0
                                    else list(range(2 * NPAIR)))
                        for ocb in ocb_list:
                            is_q = ocb < NPAIR
                            pblk = ocb % NPAIR
                            acc = psB.tile([128, 1024], F32, tag="qk")
                            for cb in range(NCB):
                                wr = pbw.tile([128, 128], F32R, tag="wr")
                                nc.sync.dma_start(wr[:], wqk_d[g, ocb, cb])
                                for n2 in range(2):
                                    nc.tensor.matmul(acc[:, n2 * 512:(n2 + 1) * 512],
                                                     wr[:], lnr[:, cb, n2 * 512:(n2 + 1) * 512],
                                                     start=(cb == 0), stop=(cb == NCB - 1))
                            if is_q:
                                nc.vector.tensor_copy(qT_g[:, pblk, :], acc[:])
                            else:
                                nc.vector.tensor_copy(kT_g[:, pblk, tsl], acc[:])
                        for sb_l in range(8):
                            sblk = tck * 8 + sb_l
                            vps = psB.tile([128, 512], F32, tag="vps")
                            for cb in range(NCB):
                                nc.tensor.matmul(vps[:], lnr[:, cb, sb_l * 128:(sb_l + 1) * 128],
                                                 vw[cb][:], start=(cb == 0), stop=(cb == NCB - 1))
                            nc.vector.tensor_copy(v_aug[:, sblk, :, 0:64], vps[:])

                # ---- Phase C: attention for this group ----
                with tc.tile_pool(name="pct", bufs=2) as pct, \
                     tc.tile_pool(name="psS", bufs=2, space="PSUM") as psS, \
                     tc.tile_pool(name="psY", bufs=2, space="PSUM") as psY:
                    for qc in range(2):
                        qsl = slice(qc * 512, (qc + 1) * 512)
                        trip = 12 + 4 * qc
                        vbase = 0 if qc == 0 else 12
                        for pair in range(NPAIR):
                            y01 = psY.tile([65, 1024], F32, tag="y01")
                            for j in range(trip):
                                st = (j == 0)
                                sp = (j == trip - 1)
                                jsl = slice(j * 128, (j + 1) * 128)
                                s01 = psS.tile([128, 1024], F32, tag="s01")
                                nc.tensor.matmul(s01[:, 0:512], kT_g[0:64, pair, jsl],
                                                 qT_g[0:64, pair, qsl], start=True, stop=True,
                                                 tile_position=(0, 0))
                                nc.tensor.matmul(s01[:, 512:1024], kT_g[64:128, pair, jsl],
                                                 qT_g[64:128, pair, qsl], start=True, stop=True,
                                                 tile_position=(64, 0))
                                p01 = pct.tile([128, 1024], F32, tag="p01")
                                nc.scalar.activation(p01[:], s01[:], AF.Exp, scale=0.125)
                                p01r = pct.tile([128, 1024], F32R, tag="p01r")
                                m2 = masks[:, vbase + j:vbase + j + 1, :].broadcast_to(
                                    [128, 2, 512])
                                nc.vector.tensor_tensor(p01r[:], p01[:], m2, OP.mult)
                                nc.tensor.matmul(y01[:, 0:512], v_aug[:, j, 2 * pair, :],
                                                 p01r[:, 0:512], start=st, stop=sp)
                                nc.tensor.matmul(y01[:, 512:1024], v_aug[:, j, 2 * pair + 1, :],
                                                 p01r[:, 512:1024], start=st, stop=sp)
                            rec01 = pct.tile([1, 1024], F32, tag="rec01")
                            nc.vector.reciprocal(rec01[:], y01[64:65, :])
                            rb01 = pct.tile([64, 1024], F32, tag="rb01")
                            nc.gpsimd.partition_broadcast(rb01[:], rec01[:], channels=64)
                            blk = g * NPAIR + pair
                            yn = pct.tile([128, 512], F32R, tag="yn")
                            nc.vector.tensor_tensor(yn[0:64, :], y01[0:64, 0:512],
                                                    rb01[:, 0:512], OP.mult)
                            nc.vector.tensor_tensor(yn[64:128, :], y01[0:64, 512:1024],
                                                    rb01[:, 512:1024], OP.mult)
                            nc.sync.dma_start(y_dram[blk * 128:(blk + 1) * 128, qsl],
                                              yn[:].bitcast(F32))

        # ============ Phase D: out-projection + residual + LN2 ============
        with tc.tile_pool(name="pd", bufs=1) as pd:
            x1 = pd.tile([128, NCB, TOK], F32)
            with tc.tile_pool(name="pdw", bufs=6) as pdw, \
                 tc.tile_pool(name="pdt", bufs=2) as pdt, \
                 tc.tile_pool(name="psD", bufs=2, space="PSUM") as psD:
                for ocb in range(NCB):
                    acc = psD.tile([128, 1024], F32, tag="proj")
                    for cblk in range(NCB):
                        wr = pdw.tile([128, 128], F32R, tag="wr")
                        nc.sync.dma_start(wr[:], wo_d[ocb, cblk])
                        for n2 in range(2):
                            yt = pdt.tile([128, 512], F32R, tag="yt")
                            nc.sync.dma_start(yt[:].bitcast(F32),
                                              y_dram[cblk * 128:(cblk + 1) * 128,
                                                     n2 * 512:(n2 + 1) * 512])
                            nc.tensor.matmul(acc[:, n2 * 512:(n2 + 1) * 512], wr[:], yt[:],
                                             start=(cblk == 0), stop=(cblk == NCB - 1))
                    xf = pdt.tile([128, TOK], F32, tag="xres")
                    # own tokens are chunk 1 after host rotation
                    nc.sync.dma_start(xf[:], xT_d[ocb * 128:(ocb + 1) * 128, TOK:T])
                    nc.vector.tensor_add(x1[:, ocb, :], acc[:], xf[:])

            # LN2 stats + normalize
            ln2r = pd.tile([128, NCB, TOK], F32R)
            with tc.tile_pool(name="pet", bufs=3) as pet, \
                 tc.tile_pool(name="psE", bufs=2, space="PSUM") as psE:
                m2v = pd.tile([1, TOK], F32)
                v2 = pd.tile([1, TOK], F32)
                for tcks in range(TOK // 512):
                    sl = slice(tcks * 512, (tcks + 1) * 512)
                    sum_ps = psE.tile([1, 512], F32, tag="sum")
                    sq_ps = psE.tile([1, 512], F32, tag="sq")
                    for cb in range(NCB):
                        xr = pet.tile([128, 512], F32R, tag="xr")
                        nc.vector.tensor_copy(xr[:], x1[:, cb, sl])
                        x2 = pet.tile([128, 512], F32R, tag="x2")
                        nc.vector.tensor_tensor(x2[:], x1[:, cb, sl], x1[:, cb, sl], OP.mult)
                        nc.tensor.matmul(sum_ps[:], ones_r[:, 0:1], xr[:],
                                         start=(cb == 0), stop=(cb == NCB - 1))
                        nc.tensor.matmul(sq_ps[:], ones_r[:, 0:1], x2[:],
                                         start=(cb == 0), stop=(cb == NCB - 1))
                    nc.scalar.mul(m2v[:, sl], sum_ps[:], 1.0 / C)
                    nc.scalar.mul(v2[:, sl], sq_ps[:], 1.0 / C)
                ms2 = pd.tile([1, TOK], F32)
                nc.vector.tensor_mul(ms2[:], m2v[:], m2v[:])
                nc.vector.tensor_sub(v2[:], v2[:], ms2[:])
                nc.scalar.activation(v2[:], v2[:], AF.Sqrt, bias=eps_t[0:1, :])
                nc.vector.reciprocal(v2[:], v2[:])
                mean2_b = pd.tile([128, TOK], F32)
                rstd2_b = pd.tile([128, TOK], F32)
                nc.gpsimd.partition_broadcast(mean2_b[:], m2v[:])
                nc.gpsimd.partition_broadcast(rstd2_b[:], v2[:])
                for cb in range(NCB):
                    xc = pet.tile([128, TOK], F32, tag="xc2")
                    nc.vector.tensor_sub(xc[:], x1[:, cb, :], mean2_b[:])
                    nc.vector.tensor_tensor(ln2r[:, cb, :], xc[:], rstd2_b[:], OP.mult)

            # ============ Phase E: MLP ============
            with tc.tile_pool(name="ph", bufs=1) as ph, \
                 tc.tile_pool(name="pew", bufs=8) as pew, \
                 tc.tile_pool(name="peo", bufs=3) as peo, \
                 tc.tile_pool(name="psF", bufs=2, space="PSUM") as psF:
                for tcks in range(TOK // 512):
                    sl = slice(tcks * 512, (tcks + 1) * 512)
                    h_r = ph.tile([128, NFB, 512], F32R, tag="h")
                    for fb in range(NFB):
                        fc = psF.tile([128, 512], F32, tag="fc")
                        for cb in range(NCB):
                            wr = pew.tile([128, 128], F32R, tag="fwr")
                            nc.sync.dma_start(wr[:], wfc_d[fb, cb])
                            nc.tensor.matmul(fc[:], wr[:], ln2r[:, cb, sl],
                                             start=(cb == 0), stop=(cb == NCB - 1))
                        nc.scalar.activation(h_r[:, fb, :], fc[:], AF.Gelu)
                    for ocb in range(NCB):
                        acc = psF.tile([128, 512], F32, tag="pacc")
                        for fb in range(NFB):
                            wr = pew.tile([128, 128], F32R, tag="pwr")
                            nc.sync.dma_start(wr[:], wproj_d[ocb, fb])
                            nc.tensor.matmul(acc[:], wr[:], h_r[:, fb, :],
                                             start=(fb == 0), stop=(fb == NFB - 1))
                        of = peo.tile([128, 512], F32, tag="of")
                        nc.vector.tensor_add(of[:], acc[:], x1[:, ocb, sl])
                        nc.sync.dma_start(out_d[ocb * 128:(ocb + 1) * 128, sl], of[:])

    nc.compile()
    return nc


def _round_f32r(x):
    """fp32 -> float32r bit pattern: RNE to 11 explicit mantissa bits."""
    u = x.view(np.uint32).astype(np.uint64)
    bias = ((u >> 12) & 1) + (1 << 11) - 1
    u = (u + bias) >> 12 << 12
    return (u & 0xFFFFFFFF).astype(np.uint32).view(np.float32)


def _prep_weights(g1, w_qkv, w_o, g2, w_fc, w_proj):
    g1 = np.asarray(g1, np.float32)
    g2 = np.asarray(g2, np.float32)
    wqkvT = np.ascontiguousarray((np.asarray(w_qkv, np.float32) * g1[None, :]).T)
    woT = np.ascontiguousarray(np.asarray(w_o, np.float32).T)
    wfcT = np.ascontiguousarray((np.asarray(w_fc, np.float32) * g2[None, :]).T)
    wprojT = np.ascontiguousarray(np.asarray(w_proj, np.float32).T)

    wqk = np.empty((NG, 2 * NPAIR, NCB, 128, 128), np.float32)
    wv = np.empty((NG, NCB, 128, 512), np.float32)
    for g in range(NG):
        for ocb in range(2 * NPAIR):
            col0 = (0 if ocb < NPAIR else C) + g * 512 + (ocb % NPAIR) * 128
            for cb in range(NCB):
                wqk[g, ocb, cb] = wqkvT[cb * 128:(cb + 1) * 128, col0:col0 + 128]
        for cb in range(NCB):
            wv[g, cb] = wqkvT[cb * 128:(cb + 1) * 128,
                              2 * C + g * 512:2 * C + (g + 1) * 512]
    wo = np.empty((NCB, NCB, 128, 128), np.float32)
    for ocb in range(NCB):
        for cblk in range(NCB):
            wo[ocb, cblk] = woT[cblk * 128:(cblk + 1) * 128, ocb * 128:(ocb + 1) * 128]
    wfc = np.empty((NFB, NCB, 128, 128), np.float32)
    for fb in range(NFB):
        for cb in range(NCB):
            wfc[fb, cb] = wfcT[cb * 128:(cb + 1) * 128, fb * 128:(fb + 1) * 128]
    wproj = np.empty((NCB, NFB, 128, 128), np.float32)
    for ocb in range(NCB):
        for fb in range(NFB):
            wproj[ocb, fb] = wprojT[fb * 128:(fb + 1) * 128, ocb * 128:(ocb + 1) * 128]
    return {"wqk": _round_f32r(wqk), "wv": _round_f32r(wv), "wo": _round_f32r(wo),
            "wfc": _round_f32r(wfc), "wproj": _round_f32r(wproj)}


def _prep(x, g1, w_qkv, w_o, g2, w_fc, w_proj):
    """Build the 8 per-core input maps (all host-side)."""
    x = np.asarray(x, np.float32)
    wmap = _prep_weights(g1, w_qkv, w_o, g2, w_fc, w_proj)

    in_maps = []
    for c in range(8):
        b, h = c // 2, c % 2
        # Rotate the sequence so the core's own tokens are always chunk 1
        # ([TOK:T]) of xT: h=0 swaps halves, h=1 keeps order.
        xb = x[b]
        if h == 0:
            xb = np.concatenate([xb[TOK:], xb[:TOK]], axis=0)
        xT = np.ascontiguousarray(xb.T)
        # Causal masks in ROTATED key coordinates.
        rot = (np.arange(T) + (TOK if h == 0 else 0)) % T
        gq = rot[TOK:]           # global positions of own (query) tokens
        gk = rot                 # global positions of keys in rotated order
        masks = np.zeros((NV, 128, 512), np.float32)
        idx = 0
        for qc in range(2):
            qpos = gq[qc * 512:(qc + 1) * 512]
            for j in range(12 + 4 * qc):
                kpos = gk[j * 128:(j + 1) * 128]
                masks[idx] = (kpos[:, None] <= qpos[None, :])
                idx += 1
        assert idx == NV
        in_maps.append({"xT": xT, "masks": masks.astype(ml_dtypes.bfloat16), **wmap})
    return in_maps


def kernel(x, g1, w_qkv, w_o, g2, w_fc, w_proj, _trace=False, **_tk):
    from concourse.bass_utils import run_bass_kernel_spmd
    if "nc" not in _CACHE:
        _CACHE["nc"] = _build()
    nc = _CACHE["nc"]
    in_maps = _prep(x, g1, w_qkv, w_o, g2, w_fc, w_proj)
    res = run_bass_kernel_spmd(nc, in_maps, core_ids=list(range(8)),
                               trace=_trace, **_tk)
    _CACHE["last"] = res
    out = np.empty((B, T, C), np.float32)
    for c in range(8):
        b, h = c // 2, c % 2
        out[b, h * TOK:(h + 1) * TOK, :] = res.results[c]["outT"].T
    return out


# revision 5
# speedup vs baseline: 1.8757x; 1.5825x over previous
"""Trainium2 Bass kernel for a GPT-style transformer block.

B=4, T=2048, C=1024, H=16 heads (D=64), FF=4096.
Sharding: 8 NeuronCores, core c = 2*b + h handles batch b, token half h
(queries/output tokens [h*1024, (h+1)*1024)); K/V are computed on-core over
the full sequence. One uniform SPMD program; per-core causality enters only
through data (host-rotated x and DMA'd multiplicative masks).

All matmuls run in float32r (fp32 rounded to 11 mantissa bits, RNE — ~4x
the precision of bf16 at near-bf16 PE throughput). Weights are pre-rounded
and block-packed on the host so each weight block is one contiguous DMA
burst straight into an F32R SBUF tile. Activations stay channel-major
([channel, token]) end to end: LN statistics are taken over the partition
dim with ones-matmuls, softmax runs without max subtraction (scores are
bounded), and V is augmented with a ones column so the softmax denominator
falls out of the AV matmul itself. Softmax normalization runs on GpSimd to
keep it off the VectorE critical path.
"""
import sys

sys.path.insert(0, "/opt/trn_rl_repo")

import numpy as np
import ml_dtypes
from contextlib import ExitStack

import concourse.bass as bass
import concourse.tile as tile
from concourse import bacc, mybir

F32 = mybir.dt.float32
F32R = mybir.dt.float32r
BF16 = mybir.dt.bfloat16
AF = mybir.ActivationFunctionType
OP = mybir.AluOpType

B, T, C, H, D = 4, 2048, 1024, 16, 64
FF = 4 * C
TOK = T // 2          # tokens owned per core
NCB = C // 128        # 8 channel blocks
NFB = FF // 128       # 32 ff blocks
NG = 2                # head groups
GH = H // NG          # 8 heads per group
NPAIR = GH // 2       # 4 head pairs per group
NSB = T // 128        # 16 s-blocks
NV = 12 + 16          # mask visits per head pair (qc=0: 12, qc=1: 16)

_CACHE = {}


def _build():
    nc = bacc.Bacc("TRN2", target_bir_lowering=False, debug=False, num_devices=8)

    xT_d = nc.dram_tensor("xT", [C, T], F32, kind="ExternalInput").ap()
    wqk_d = nc.dram_tensor("wqk", [NG, 2 * NPAIR, 128, NCB, 128], F32R,
                           kind="ExternalInput").ap()
    wv_d = nc.dram_tensor("wv", [NG, 128, NCB, 512], F32R, kind="ExternalInput").ap()
    wo_d = nc.dram_tensor("wo", [NCB, 128, NCB, 128], F32R, kind="ExternalInput").ap()
    wfc_d = nc.dram_tensor("wfc", [NFB, 128, NCB, 128], F32R, kind="ExternalInput").ap()
    wproj_d = nc.dram_tensor("wproj", [NCB, 2, 128, NFB // 2, 128], F32R,
                             kind="ExternalInput").ap()
    mask_d = nc.dram_tensor("masks", [128, NV, 512], BF16, kind="ExternalInput").ap()
    out_d = nc.dram_tensor("outT", [C, TOK], F32, kind="ExternalOutput").ap()
    y_dram = nc.dram_tensor("ydram", [C, TOK], F32).ap()   # internal: normalized attn y

    with tile.TileContext(nc) as tc, ExitStack() as top:
        persist = top.enter_context(tc.tile_pool(name="persist", bufs=1))
        ones_f = persist.tile([128, 128], F32)
        nc.vector.memset(ones_f[:], 1.0)
        ones_r = persist.tile([128, 128], F32R)
        nc.vector.tensor_copy(ones_r[:], ones_f[:])
        eps_t = persist.tile([128, 1], F32)
        nc.vector.memset(eps_t[:], 1e-5)

        pa = top.enter_context(tc.tile_pool(name="pa", bufs=1))  # LN1 stat vectors
        mean_r = pa.tile([1, T], F32)
        rstd_r = pa.tile([1, T], F32)

        # ============ Phase A: LN1 statistics over channels ============
        with tc.tile_pool(name="pax", bufs=3) as pax, \
             tc.tile_pool(name="psA", bufs=2, space="PSUM") as psA:
            for tck in range(T // 512):
                sl = slice(tck * 512, (tck + 1) * 512)
                sum_ps = psA.tile([1, 512], F32, tag="sum")
                sq_ps = psA.tile([1, 512], F32, tag="sq")
                for cb in range(NCB):
                    xf = pax.tile([128, 512], F32, tag="xf")
                    nc.sync.dma_start(xf[:], xT_d[cb * 128:(cb + 1) * 128, sl])
                    xr = pax.tile([128, 512], F32R, tag="xr")
                    nc.vector.tensor_copy(xr[:], xf[:])
                    x2 = pax.tile([128, 512], F32R, tag="x2")
                    nc.vector.tensor_tensor(x2[:], xf[:], xf[:], OP.mult)
                    nc.tensor.matmul(sum_ps[:], ones_r[:, 0:1], xr[:],
                                     start=(cb == 0), stop=(cb == NCB - 1))
                    nc.tensor.matmul(sq_ps[:], ones_r[:, 0:1], x2[:],
                                     start=(cb == 0), stop=(cb == NCB - 1))
                nc.scalar.mul(mean_r[:, sl], sum_ps[:], 1.0 / C)
                nc.scalar.mul(rstd_r[:, sl], sq_ps[:], 1.0 / C)
            msq = pax.tile([1, T], F32, tag="msq")
            nc.vector.tensor_mul(msq[:], mean_r[:], mean_r[:])
            nc.vector.tensor_sub(rstd_r[:], rstd_r[:], msq[:])
            nc.scalar.activation(rstd_r[:], rstd_r[:], AF.Sqrt, bias=eps_t[0:1, :])
            nc.vector.reciprocal(rstd_r[:], rstd_r[:])

        # ============ Phases B+C per head group ============
        with tc.tile_pool(name="pb", bufs=1) as pb:
            masks = pb.tile([128, NV, 512], BF16)
            nc.sync.dma_start(masks[:], mask_d)

            for g in range(NG):
                # ---- Phase B: QKV for this group ----
                kT_g = pb.tile([128, NPAIR, T], F32R, tag="kT")
                qT_g = pb.tile([128, NPAIR, TOK], F32R, tag="qT")
                v_aug = pb.tile([128, NSB, GH, 65], F32R, tag="vaug")
                nc.vector.tensor_copy(v_aug[:, :, :, 64:65], ones_r[:, 0:NSB * GH])

                with tc.tile_pool(name="pbt", bufs=1) as pbt, \
                     tc.tile_pool(name="pbx", bufs=2) as pbx, \
                     tc.tile_pool(name="pbw", bufs=3) as pbw, \
                     tc.tile_pool(name="pbv", bufs=1) as pbv, \
                     tc.tile_pool(name="psB", bufs=2, space="PSUM") as psB:
                    # v weights: one DMA per group
                    vw = pbv.tile([128, NCB, 512], F32R, tag="vw")
                    nc.sync.dma_start(vw[:], wv_d[g])
                    for tck in range(2):  # t-chunks of 1024 over the full sequence
                        tsl = slice(tck * 1024, (tck + 1) * 1024)
                        mb = pbt.tile([128, 1024], F32, tag="mb")
                        rb = pbt.tile([128, 1024], F32, tag="rb")
                        nc.gpsimd.partition_broadcast(mb[:], mean_r[:, tsl])
                        nc.gpsimd.partition_broadcast(rb[:], rstd_r[:, tsl])
                        lnr = pbt.tile([128, NCB, 1024], F32R, tag="lnr")
                        for cb in range(NCB):
                            xf = pbx.tile([128, 1024], F32, tag="xbf")
                            nc.sync.dma_start(xf[:], xT_d[cb * 128:(cb + 1) * 128, tsl])
                            nc.vector.tensor_sub(xf[:], xf[:], mb[:])
                            nc.vector.tensor_tensor(lnr[:, cb, :], xf[:], rb[:], OP.mult)
                        # k for every chunk; q only for the own-token chunk (tck==1)
                        ocb_list = (list(range(NPAIR, 2 * NPAIR)) if tck == 0
                                    else list(range(2 * NPAIR)))
                        for ocb in ocb_list:
                            is_q = ocb < NPAIR
                            pblk = ocb % NPAIR
                            wt = pbw.tile([128, NCB, 128], F32R, tag="wt")
                            nc.sync.dma_start(wt[:], wqk_d[g, ocb])
                            acc = psB.tile([128, 1024], F32, tag="qk")
                            for cb in range(NCB):
                                for n2 in range(2):
                                    nc.tensor.matmul(acc[:, n2 * 512:(n2 + 1) * 512],
                                                     wt[:, cb, :],
                                                     lnr[:, cb, n2 * 512:(n2 + 1) * 512],
                                                     start=(cb == 0), stop=(cb == NCB - 1))
                            if is_q:
                                nc.vector.tensor_copy(qT_g[:, pblk, :], acc[:])
                            else:
                                nc.vector.tensor_copy(kT_g[:, pblk, tsl], acc[:])
                        for sb_l in range(8):
                            sblk = tck * 8 + sb_l
                            vps = psB.tile([128, 512], F32, tag="vps")
                            for cb in range(NCB):
                                nc.tensor.matmul(vps[:], lnr[:, cb, sb_l * 128:(sb_l + 1) * 128],
                                                 vw[:, cb, :], start=(cb == 0), stop=(cb == NCB - 1))
                            nc.vector.tensor_copy(v_aug[:, sblk, :, 0:64], vps[:])

                # ---- Phase C: attention for this group ----
                with tc.tile_pool(name="pct", bufs=3) as pct, \
                     tc.tile_pool(name="pcn", bufs=2) as pcn, \
                     tc.tile_pool(name="psS", bufs=3, space="PSUM") as psS, \
                     tc.tile_pool(name="psY", bufs=1, space="PSUM") as psY:
                    for qc in range(2):
                        qsl = slice(qc * 512, (qc + 1) * 512)
                        trip = 12 + 4 * qc
                        vbase = 0 if qc == 0 else 12
                        for pair in range(NPAIR):
                            y01 = psY.tile([65, 1024], F32, tag="y01")
                            for j in range(trip):
                                st = (j == 0)
                                sp = (j == trip - 1)
                                jsl = slice(j * 128, (j + 1) * 128)
                                s01 = psS.tile([128, 1024], F32, tag="s01")
                                nc.tensor.matmul(s01[:, 0:512], kT_g[0:64, pair, jsl],
                                                 qT_g[0:64, pair, qsl], start=True, stop=True,
                                                 tile_position=(0, 0))
                                nc.tensor.matmul(s01[:, 512:1024], kT_g[64:128, pair, jsl],
                                                 qT_g[64:128, pair, qsl], start=True, stop=True,
                                                 tile_position=(64, 0))
                                p01 = pct.tile([128, 1024], F32, tag="p01")
                                nc.scalar.activation(p01[:], s01[:], AF.Exp, scale=0.125)
                                p01r = pct.tile([128, 1024], F32R, tag="p01r")
                                m2 = masks[:, vbase + j:vbase + j + 1, :].broadcast_to(
                                    [128, 2, 512])
                                nc.vector.tensor_tensor(p01r[:], p01[:], m2, OP.mult)
                                nc.tensor.matmul(y01[:, 0:512], v_aug[:, j, 2 * pair, :],
                                                 p01r[:, 0:512], start=st, stop=sp)
                                nc.tensor.matmul(y01[:, 512:1024], v_aug[:, j, 2 * pair + 1, :],
                                                 p01r[:, 512:1024], start=st, stop=sp)
                            # Normalize off the DVE critical path: DVE only
                            # evacuates PSUM; GpSimd does broadcast + division.
                            ysb = pcn.tile([65, 1024], F32, tag="ysb")
                            nc.vector.tensor_copy(ysb[:], y01[:])
                            rec01 = pcn.tile([1, 1024], F32, tag="rec01")
                            nc.vector.reciprocal(rec01[:], y01[64:65, :])
                            rb01 = pcn.tile([64, 1024], F32, tag="rb01")
                            nc.gpsimd.partition_broadcast(rb01[:], rec01[:], channels=64)
                            yn0 = pcn.tile([64, 512], F32, tag="yn0")
                            yn1 = pcn.tile([64, 512], F32, tag="yn1")
                            nc.gpsimd.tensor_tensor(yn0[:], ysb[0:64, 0:512],
                                                    rb01[:, 0:512], OP.mult)
                            nc.gpsimd.tensor_tensor(yn1[:], ysb[0:64, 512:1024],
                                                    rb01[:, 512:1024], OP.mult)
                            blk = g * NPAIR + pair
                            nc.sync.dma_start(y_dram[blk * 128:blk * 128 + 64, qsl], yn0[:])
                            nc.sync.dma_start(y_dram[blk * 128 + 64:blk * 128 + 128, qsl],
                                              yn1[:])

        # ============ Phase D: out-projection + residual + LN2 ============
        with tc.tile_pool(name="pd", bufs=1) as pd:
            x1 = pd.tile([128, NCB, TOK], F32)
            with tc.tile_pool(name="pdw", bufs=2) as pdw, \
                 tc.tile_pool(name="pdt", bufs=2) as pdt, \
                 tc.tile_pool(name="pdy", bufs=1) as pdy, \
                 tc.tile_pool(name="psD", bufs=2, space="PSUM") as psD:
                y_sb = pdy.tile([128, NCB, TOK], F32R, tag="ysb")
                for cblk in range(NCB):
                    yf = pdt.tile([128, TOK], F32, tag="yf")
                    nc.sync.dma_start(yf[:], y_dram[cblk * 128:(cblk + 1) * 128, :])
                    nc.vector.tensor_copy(y_sb[:, cblk, :], yf[:])
                for ocb in range(NCB):
                    wt = pdw.tile([128, NCB, 128], F32R, tag="wt")
                    nc.sync.dma_start(wt[:], wo_d[ocb])
                    acc = psD.tile([128, 1024], F32, tag="proj")
                    for cblk in range(NCB):
                        for n2 in range(2):
                            nc.tensor.matmul(acc[:, n2 * 512:(n2 + 1) * 512], wt[:, cblk, :],
                                             y_sb[:, cblk, n2 * 512:(n2 + 1) * 512],
                                             start=(cblk == 0), stop=(cblk == NCB - 1))
                    xf = pdt.tile([128, TOK], F32, tag="xres")
                    # own tokens are chunk 1 after host rotation
                    nc.sync.dma_start(xf[:], xT_d[ocb * 128:(ocb + 1) * 128, TOK:T])
                    nc.vector.tensor_add(x1[:, ocb, :], acc[:], xf[:])

            # LN2 stats + normalize
            ln2r = pd.tile([128, NCB, TOK], F32R)
            with tc.tile_pool(name="pet", bufs=3) as pet, \
                 tc.tile_pool(name="psE", bufs=2, space="PSUM") as psE:
                m2v = pd.tile([1, TOK], F32)
                v2 = pd.tile([1, TOK], F32)
                for tcks in range(TOK // 512):
                    sl = slice(tcks * 512, (tcks + 1) * 512)
                    sum_ps = psE.tile([1, 512], F32, tag="sum")
                    sq_ps = psE.tile([1, 512], F32, tag="sq")
                    for cb in range(NCB):
                        xr = pet.tile([128, 512], F32R, tag="xr")
                        nc.vector.tensor_copy(xr[:], x1[:, cb, sl])
                        x2 = pet.tile([128, 512], F32R, tag="x2")
                        nc.vector.tensor_tensor(x2[:], x1[:, cb, sl], x1[:, cb, sl], OP.mult)
                        nc.tensor.matmul(sum_ps[:], ones_r[:, 0:1], xr[:],
                                         start=(cb == 0), stop=(cb == NCB - 1))
                        nc.tensor.matmul(sq_ps[:], ones_r[:, 0:1], x2[:],
                                         start=(cb == 0), stop=(cb == NCB - 1))
                    nc.scalar.mul(m2v[:, sl], sum_ps[:], 1.0 / C)
                    nc.scalar.mul(v2[:, sl], sq_ps[:], 1.0 / C)
                ms2 = pd.tile([1, TOK], F32)
                nc.vector.tensor_mul(ms2[:], m2v[:], m2v[:])
                nc.vector.tensor_sub(v2[:], v2[:], ms2[:])
                nc.scalar.activation(v2[:], v2[:], AF.Sqrt, bias=eps_t[0:1, :])
                nc.vector.reciprocal(v2[:], v2[:])
                mean2_b = pd.tile([128, TOK], F32)
                rstd2_b = pd.tile([128, TOK], F32)
                nc.gpsimd.partition_broadcast(mean2_b[:], m2v[:])
                nc.gpsimd.partition_broadcast(rstd2_b[:], v2[:])
                for cb in range(NCB):
                    xc = pet.tile([128, TOK], F32, tag="xc2")
                    nc.vector.tensor_sub(xc[:], x1[:, cb, :], mean2_b[:])
                    nc.vector.tensor_tensor(ln2r[:, cb, :], xc[:], rstd2_b[:], OP.mult)

            # ============ Phase E: MLP ============
            with tc.tile_pool(name="ph", bufs=1) as ph, \
                 tc.tile_pool(name="pew", bufs=3) as pew, \
                 tc.tile_pool(name="pepw", bufs=2) as pepw, \
                 tc.tile_pool(name="peo", bufs=3) as peo, \
                 tc.tile_pool(name="psF", bufs=2, space="PSUM") as psF:
                for tcks in range(TOK // 512):
                    sl = slice(tcks * 512, (tcks + 1) * 512)
                    h_r = ph.tile([128, NFB, 512], F32R, tag="h")
                    for fb in range(NFB):
                        wt = pew.tile([128, NCB, 128], F32R, tag="fwt")
                        nc.sync.dma_start(wt[:], wfc_d[fb])
                        fc = psF.tile([128, 512], F32, tag="fc")
                        for cb in range(NCB):
                            nc.tensor.matmul(fc[:], wt[:, cb, :], ln2r[:, cb, sl],
                                             start=(cb == 0), stop=(cb == NCB - 1))
                        nc.scalar.activation(h_r[:, fb, :], fc[:], AF.Gelu)
                    for ocb in range(NCB):
                        acc = psF.tile([128, 512], F32, tag="pacc")
                        for fh in range(2):
                            wt = pepw.tile([128, NFB // 2, 128], F32R, tag="pwt")
                            nc.sync.dma_start(wt[:], wproj_d[ocb, fh])
                            for fi in range(NFB // 2):
                                fb = fh * (NFB // 2) + fi
                                nc.tensor.matmul(acc[:], wt[:, fi, :], h_r[:, fb, :],
                                                 start=(fb == 0), stop=(fb == NFB - 1))
                        of = peo.tile([128, 512], F32, tag="of")
                        nc.vector.tensor_add(of[:], acc[:], x1[:, ocb, sl])
                        nc.sync.dma_start(out_d[ocb * 128:(ocb + 1) * 128, sl], of[:])

    nc.compile()
    return nc


def _round_f32r(x):
    """fp32 -> float32r bit pattern: RNE to 11 explicit mantissa bits."""
    u = x.view(np.uint32).astype(np.uint64)
    bias = ((u >> 12) & 1) + (1 << 11) - 1
    u = (u + bias) >> 12 << 12
    return (u & 0xFFFFFFFF).astype(np.uint32).view(np.float32)


def _prep_weights(g1, w_qkv, w_o, g2, w_fc, w_proj):
    g1 = np.asarray(g1, np.float32)
    g2 = np.asarray(g2, np.float32)
    wqkvT = np.ascontiguousarray((np.asarray(w_qkv, np.float32) * g1[None, :]).T)
    woT = np.ascontiguousarray(np.asarray(w_o, np.float32).T)
    wfcT = np.ascontiguousarray((np.asarray(w_fc, np.float32) * g2[None, :]).T)
    wprojT = np.ascontiguousarray(np.asarray(w_proj, np.float32).T)

    # wqk[g, ocb, r, cb, f] = wqkvT[cb*128 + r, col0(g, ocb) + f]
    wqk = np.empty((NG, 2 * NPAIR, 128, NCB, 128), np.float32)
    wv = np.empty((NG, 128, NCB, 512), np.float32)
    for g in range(NG):
        for ocb in range(2 * NPAIR):
            col0 = (0 if ocb < NPAIR else C) + g * 512 + (ocb % NPAIR) * 128
            blk = wqkvT[:, col0:col0 + 128].reshape(NCB, 128, 128)  # [cb, r, f]
            wqk[g, ocb] = blk.transpose(1, 0, 2)
        vblk = wqkvT[:, 2 * C + g * 512:2 * C + (g + 1) * 512].reshape(NCB, 128, 512)
        wv[g] = vblk.transpose(1, 0, 2)
    wo = np.empty((NCB, 128, NCB, 128), np.float32)
    for ocb in range(NCB):
        blk = woT[:, ocb * 128:(ocb + 1) * 128].reshape(NCB, 128, 128)
        wo[ocb] = blk.transpose(1, 0, 2)
    wfc = np.empty((NFB, 128, NCB, 128), np.float32)
    for fb in range(NFB):
        blk = wfcT[:, fb * 128:(fb + 1) * 128].reshape(NCB, 128, 128)
        wfc[fb] = blk.transpose(1, 0, 2)
    wproj = np.empty((NCB, 2, 128, NFB // 2, 128), np.float32)
    for ocb in range(NCB):
        blk = wprojT[:, ocb * 128:(ocb + 1) * 128].reshape(NFB, 128, 128)  # [fb, r, f]
        for fh in range(2):
            wproj[ocb, fh] = blk[fh * (NFB // 2):(fh + 1) * (NFB // 2)].transpose(1, 0, 2)
    return {"wqk": _round_f32r(wqk), "wv": _round_f32r(wv), "wo": _round_f32r(wo),
            "wfc": _round_f32r(wfc), "wproj": _round_f32r(wproj)}


def _prep(x, g1, w_qkv, w_o, g2, w_fc, w_proj):
    """Build the 8 per-core input maps (all host-side)."""
    x = np.asarray(x, np.float32)
    wmap = _prep_weights(g1, w_qkv, w_o, g2, w_fc, w_proj)

    in_maps = []
    for c in range(8):
        b, h = c // 2, c % 2
        # Rotate the sequence so the core's own tokens are always chunk 1
        # ([TOK:T]) of xT: h=0 swaps halves, h=1 keeps order.
        xb = x[b]
        if h == 0:
            xb = np.concatenate([xb[TOK:], xb[:TOK]], axis=0)
        xT = np.ascontiguousarray(xb.T)
        # Causal masks in ROTATED key coordinates, packed [row, visit, 512].
        rot = (np.arange(T) + (TOK if h == 0 else 0)) % T
        gq = rot[TOK:]           # global positions of own (query) tokens
        gk = rot                 # global positions of keys in rotated order
        masks = np.zeros((NV, 128, 512), np.float32)
        idx = 0
        for qc in range(2):
            qpos = gq[qc * 512:(qc + 1) * 512]
            for j in range(12 + 4 * qc):
                kpos = gk[j * 128:(j + 1) * 128]
                masks[idx] = (kpos[:, None] <= qpos[None, :])
                idx += 1
        assert idx == NV
        masks = np.ascontiguousarray(masks.transpose(1, 0, 2))  # [128, NV, 512]
        in_maps.append({"xT": xT, "masks": masks.astype(ml_dtypes.bfloat16), **wmap})
    return in_maps


def kernel(x, g1, w_qkv, w_o, g2, w_fc, w_proj, _trace=False, **_tk):
    from concourse.bass_utils import run_bass_kernel_spmd
    if "nc" not in _CACHE:
        _CACHE["nc"] = _build()
    nc = _CACHE["nc"]
    in_maps = _prep(x, g1, w_qkv, w_o, g2, w_fc, w_proj)
    res = run_bass_kernel_spmd(nc, in_maps, core_ids=list(range(8)),
                               trace=_trace, **_tk)
    _CACHE["last"] = res
    out = np.empty((B, T, C), np.float32)
    for c in range(8):
        b, h = c // 2, c % 2
        out[b, h * TOK:(h + 1) * TOK, :] = res.results[c]["outT"].T
    return out


# revision 6
# speedup vs baseline: 1.9966x; 1.0645x over previous
"""Trainium2 Bass kernel for a GPT-style transformer block.

B=4, T=2048, C=1024, H=16 heads (D=64), FF=4096.
Sharding: 8 NeuronCores, core c = 2*b + h handles batch b, token half h
(queries/output tokens [h*1024, (h+1)*1024)); K/V are computed on-core over
the full sequence. One uniform SPMD program; per-core causality enters only
through data (host-rotated x and DMA'd multiplicative masks).

All matmuls run in float32r (fp32 rounded to 11 mantissa bits, RNE — ~4x
the precision of bf16 at near-bf16 PE throughput). Weights are pre-rounded
and block-packed on the host so each weight block is one contiguous DMA
burst straight into an F32R SBUF tile. Activations stay channel-major
([channel, token]) end to end: LN statistics are taken over the partition
dim with ones-matmuls, softmax runs without max subtraction (scores are
bounded), and V is augmented with a ones column so the softmax denominator
falls out of the AV matmul itself. Softmax normalization runs on GpSimd to
keep it off the VectorE critical path.
"""
import sys

sys.path.insert(0, "/opt/trn_rl_repo")

import numpy as np
import ml_dtypes
from contextlib import ExitStack

import concourse.bass as bass
import concourse.tile as tile
from concourse import bacc, mybir

F32 = mybir.dt.float32
F32R = mybir.dt.float32r
BF16 = mybir.dt.bfloat16
AF = mybir.ActivationFunctionType
OP = mybir.AluOpType

B, T, C, H, D = 4, 2048, 1024, 16, 64
FF = 4 * C
TOK = T // 2          # tokens owned per core
NCB = C // 128        # 8 channel blocks
NFB = FF // 128       # 32 ff blocks
NG = 2                # head groups
GH = H // NG          # 8 heads per group
NPAIR = GH // 2       # 4 head pairs per group
NSB = T // 128        # 16 s-blocks
NV = 8                # diagonal mask visits (4 per q-chunk, same j's on every core)

_CACHE = {}


def _build():
    nc = bacc.Bacc("TRN2", target_bir_lowering=False, debug=False, num_devices=8)

    xT_d = nc.dram_tensor("xT", [C, T], F32, kind="ExternalInput").ap()
    wqk_d = nc.dram_tensor("wqk", [NG, 2 * NPAIR, 128, NCB, 128], F32R,
                           kind="ExternalInput").ap()
    wv_d = nc.dram_tensor("wv", [NG, 128, NCB, 512], F32R, kind="ExternalInput").ap()
    wo_d = nc.dram_tensor("wo", [NCB, 128, NCB, 128], F32R, kind="ExternalInput").ap()
    wfc_d = nc.dram_tensor("wfc", [NFB, 128, NCB, 128], F32R, kind="ExternalInput").ap()
    wproj_d = nc.dram_tensor("wproj", [NCB, 2, 128, NFB // 2, 128], F32R,
                             kind="ExternalInput").ap()
    mask_d = nc.dram_tensor("masks", [128, NV, 512], BF16, kind="ExternalInput").ap()
    vmask_d = nc.dram_tensor("vmask", [128, NSB], F32, kind="ExternalInput").ap()
    out_d = nc.dram_tensor("outT", [C, TOK], F32, kind="ExternalOutput").ap()
    y_dram = nc.dram_tensor("ydram", [C, TOK], F32).ap()   # internal: normalized attn y

    with tile.TileContext(nc) as tc, ExitStack() as top:
        persist = top.enter_context(tc.tile_pool(name="persist", bufs=1))
        ones_f = persist.tile([128, 128], F32)
        nc.vector.memset(ones_f[:], 1.0)
        ones_r = persist.tile([128, 128], F32R)
        nc.vector.tensor_copy(ones_r[:], ones_f[:])
        eps_t = persist.tile([128, 1], F32)
        nc.vector.memset(eps_t[:], 1e-5)

        pa = top.enter_context(tc.tile_pool(name="pa", bufs=1))  # LN1 stat vectors
        mean_r = pa.tile([1, T], F32)
        rstd_r = pa.tile([1, T], F32)

        # ============ Phase A: LN1 statistics over channels ============
        with tc.tile_pool(name="pax", bufs=3) as pax, \
             tc.tile_pool(name="psA", bufs=2, space="PSUM") as psA:
            for tck in range(T // 512):
                sl = slice(tck * 512, (tck + 1) * 512)
                sum_ps = psA.tile([1, 512], F32, tag="sum")
                sq_ps = psA.tile([1, 512], F32, tag="sq")
                for cb in range(NCB):
                    xf = pax.tile([128, 512], F32, tag="xf")
                    nc.sync.dma_start(xf[:], xT_d[cb * 128:(cb + 1) * 128, sl])
                    xr = pax.tile([128, 512], F32R, tag="xr")
                    nc.vector.tensor_copy(xr[:], xf[:])
                    x2 = pax.tile([128, 512], F32R, tag="x2")
                    nc.vector.tensor_tensor(x2[:], xf[:], xf[:], OP.mult)
                    nc.tensor.matmul(sum_ps[:], ones_r[:, 0:1], xr[:],
                                     start=(cb == 0), stop=(cb == NCB - 1))
                    nc.tensor.matmul(sq_ps[:], ones_r[:, 0:1], x2[:],
                                     start=(cb == 0), stop=(cb == NCB - 1))
                nc.scalar.mul(mean_r[:, sl], sum_ps[:], 1.0 / C)
                nc.scalar.mul(rstd_r[:, sl], sq_ps[:], 1.0 / C)
                msq = pax.tile([1, 512], F32, tag="msq")
                nc.vector.tensor_mul(msq[:], mean_r[:, sl], mean_r[:, sl])
                nc.vector.tensor_sub(rstd_r[:, sl], rstd_r[:, sl], msq[:])
                nc.scalar.activation(rstd_r[:, sl], rstd_r[:, sl], AF.Sqrt,
                                     bias=eps_t[0:1, :])
                nc.vector.reciprocal(rstd_r[:, sl], rstd_r[:, sl])

        # ============ Phases B+C per head group ============
        with tc.tile_pool(name="pb", bufs=1) as pb:
            masks = pb.tile([128, NV, 512], BF16)
            nc.sync.dma_start(masks[:], mask_d)
            vmask = pb.tile([128, NSB], F32)
            nc.sync.dma_start(vmask[:], vmask_d)

            for g in range(NG):
                # ---- Phase B: QKV for this group ----
                kT_g = pb.tile([128, NPAIR, T], F32R, tag="kT")
                qT_g = pb.tile([128, NPAIR, TOK], F32R, tag="qT")
                v_aug = pb.tile([128, NSB, GH, 65], F32R, tag="vaug")
                for sblk in range(NSB):
                    nc.vector.tensor_copy(
                        v_aug[:, sblk, :, 64:65],
                        vmask[:, sblk:sblk + 1].broadcast_to([128, GH, 1]))

                with tc.tile_pool(name="pbt", bufs=1) as pbt, \
                     tc.tile_pool(name="pbx", bufs=2) as pbx, \
                     tc.tile_pool(name="pbw", bufs=3) as pbw, \
                     tc.tile_pool(name="pbv", bufs=1) as pbv, \
                     tc.tile_pool(name="psB", bufs=2, space="PSUM") as psB:
                    # v weights: one DMA per group
                    vw = pbv.tile([128, NCB, 512], F32R, tag="vw")
                    nc.sync.dma_start(vw[:], wv_d[g])
                    for tck in range(2):  # t-chunks of 1024 over the full sequence
                        tsl = slice(tck * 1024, (tck + 1) * 1024)
                        mb = pbt.tile([128, 1024], F32, tag="mb")
                        rb = pbt.tile([128, 1024], F32, tag="rb")
                        nc.gpsimd.partition_broadcast(mb[:], mean_r[:, tsl])
                        nc.gpsimd.partition_broadcast(rb[:], rstd_r[:, tsl])
                        lnr = pbt.tile([128, NCB, 1024], F32R, tag="lnr")
                        for cb in range(NCB):
                            xf = pbx.tile([128, 1024], F32, tag="xbf")
                            nc.sync.dma_start(xf[:], xT_d[cb * 128:(cb + 1) * 128, tsl])
                            nc.vector.tensor_sub(xf[:], xf[:], mb[:])
                            nc.vector.tensor_tensor(lnr[:, cb, :], xf[:], rb[:], OP.mult)
                        # k for every chunk; q only for the own-token chunk (tck==1)
                        ocb_list = (list(range(NPAIR, 2 * NPAIR)) if tck == 0
                                    else list(range(2 * NPAIR)))
                        for ocb in ocb_list:
                            is_q = ocb < NPAIR
                            pblk = ocb % NPAIR
                            wt = pbw.tile([128, NCB, 128], F32R, tag="wt")
                            nc.sync.dma_start(wt[:], wqk_d[g, ocb])
                            acc = psB.tile([128, 1024], F32, tag="qk")
                            for cb in range(NCB):
                                for n2 in range(2):
                                    nc.tensor.matmul(acc[:, n2 * 512:(n2 + 1) * 512],
                                                     wt[:, cb, :],
                                                     lnr[:, cb, n2 * 512:(n2 + 1) * 512],
                                                     start=(cb == 0), stop=(cb == NCB - 1))
                            if is_q:
                                nc.vector.tensor_copy(qT_g[:, pblk, :], acc[:])
                            else:
                                nc.vector.tensor_copy(kT_g[:, pblk, tsl], acc[:])
                        for sb_l in range(8):
                            sblk = tck * 8 + sb_l
                            vps = psB.tile([128, 512], F32, tag="vps")
                            for cb in range(NCB):
                                nc.tensor.matmul(vps[:], lnr[:, cb, sb_l * 128:(sb_l + 1) * 128],
                                                 vw[:, cb, :], start=(cb == 0), stop=(cb == NCB - 1))
                            nc.vector.tensor_scalar_mul(v_aug[:, sblk, :, 0:64], vps[:],
                                                        vmask[:, sblk:sblk + 1])

                # ---- Phase C: attention for this group ----
                with tc.tile_pool(name="pct", bufs=3) as pct, \
                     tc.tile_pool(name="pcn", bufs=2) as pcn, \
                     tc.tile_pool(name="psS", bufs=3, space="PSUM") as psS, \
                     tc.tile_pool(name="psY", bufs=1, space="PSUM") as psY:
                    for qc in range(2):
                        qsl = slice(qc * 512, (qc + 1) * 512)
                        trip = 12 + 4 * qc
                        for pair in range(NPAIR):
                            y01 = psY.tile([65, 1024], F32, tag="y01")
                            for j in range(trip):
                                st = (j == 0)
                                sp = (j == trip - 1)
                                jsl = slice(j * 128, (j + 1) * 128)
                                s01 = psS.tile([128, 1024], F32, tag="s01")
                                nc.tensor.matmul(s01[:, 0:512], kT_g[0:64, pair, jsl],
                                                 qT_g[0:64, pair, qsl], start=True, stop=True,
                                                 tile_position=(0, 0))
                                nc.tensor.matmul(s01[:, 512:1024], kT_g[64:128, pair, jsl],
                                                 qT_g[64:128, pair, qsl], start=True, stop=True,
                                                 tile_position=(64, 0))
                                p01r = pct.tile([128, 1024], F32R, tag="p01r")
                                if j >= trip - 4:  # diagonal: mask needed
                                    p01 = pct.tile([128, 1024], F32, tag="p01")
                                    nc.scalar.activation(p01[:], s01[:], AF.Exp, scale=0.125)
                                    vi = qc * 4 + (j - (trip - 4))
                                    m2 = masks[:, vi:vi + 1, :].broadcast_to([128, 2, 512])
                                    nc.vector.tensor_tensor(p01r[:], p01[:], m2, OP.mult)
                                else:
                                    nc.scalar.activation(p01r[:], s01[:], AF.Exp, scale=0.125)
                                nc.tensor.matmul(y01[:, 0:512], v_aug[:, j, 2 * pair, :],
                                                 p01r[:, 0:512], start=st, stop=sp)
                                nc.tensor.matmul(y01[:, 512:1024], v_aug[:, j, 2 * pair + 1, :],
                                                 p01r[:, 512:1024], start=st, stop=sp)
                            # Normalize off the DVE critical path: DVE only
                            # evacuates PSUM; GpSimd does broadcast + division.
                            ysb = pcn.tile([65, 1024], F32, tag="ysb")
                            nc.vector.tensor_copy(ysb[:], y01[:])
                            rec01 = pcn.tile([1, 1024], F32, tag="rec01")
                            nc.vector.reciprocal(rec01[:], y01[64:65, :])
                            rb01 = pcn.tile([64, 1024], F32, tag="rb01")
                            nc.gpsimd.partition_broadcast(rb01[:], rec01[:], channels=64)
                            yn0 = pcn.tile([64, 512], F32, tag="yn0")
                            yn1 = pcn.tile([64, 512], F32, tag="yn1")
                            nc.gpsimd.tensor_tensor(yn0[:], ysb[0:64, 0:512],
                                                    rb01[:, 0:512], OP.mult)
                            nc.gpsimd.tensor_tensor(yn1[:], ysb[0:64, 512:1024],
                                                    rb01[:, 512:1024], OP.mult)
                            blk = g * NPAIR + pair
                            nc.sync.dma_start(y_dram[blk * 128:blk * 128 + 64, qsl], yn0[:])
                            nc.sync.dma_start(y_dram[blk * 128 + 64:blk * 128 + 128, qsl],
                                              yn1[:])

        # ============ Phase D: out-projection + residual + LN2 ============
        with tc.tile_pool(name="pd", bufs=1) as pd:
            x1 = pd.tile([128, NCB, TOK], F32)
            with tc.tile_pool(name="pdw", bufs=2) as pdw, \
                 tc.tile_pool(name="pdt", bufs=2) as pdt, \
                 tc.tile_pool(name="pdy", bufs=1) as pdy, \
                 tc.tile_pool(name="psD", bufs=2, space="PSUM") as psD:
                y_sb = pdy.tile([128, NCB, TOK], F32R, tag="ysb")
                for cblk in range(NCB):
                    yf = pdt.tile([128, TOK], F32, tag="yf")
                    nc.sync.dma_start(yf[:], y_dram[cblk * 128:(cblk + 1) * 128, :])
                    nc.vector.tensor_copy(y_sb[:, cblk, :], yf[:])
                for ocb in range(NCB):
                    wt = pdw.tile([128, NCB, 128], F32R, tag="wt")
                    nc.sync.dma_start(wt[:], wo_d[ocb])
                    acc = psD.tile([128, 1024], F32, tag="proj")
                    for cblk in range(NCB):
                        for n2 in range(2):
                            nc.tensor.matmul(acc[:, n2 * 512:(n2 + 1) * 512], wt[:, cblk, :],
                                             y_sb[:, cblk, n2 * 512:(n2 + 1) * 512],
                                             start=(cblk == 0), stop=(cblk == NCB - 1))
                    xf = pdt.tile([128, TOK], F32, tag="xres")
                    # own tokens are chunk 1 after host rotation
                    nc.sync.dma_start(xf[:], xT_d[ocb * 128:(ocb + 1) * 128, TOK:T])
                    nc.vector.tensor_add(x1[:, ocb, :], acc[:], xf[:])

            # LN2 stats + normalize
            ln2r = pd.tile([128, NCB, TOK], F32R)
            with tc.tile_pool(name="pet", bufs=3) as pet, \
                 tc.tile_pool(name="psE", bufs=2, space="PSUM") as psE:
                m2v = pd.tile([1, TOK], F32)
                v2 = pd.tile([1, TOK], F32)
                for tcks in range(TOK // 512):
                    sl = slice(tcks * 512, (tcks + 1) * 512)
                    sum_ps = psE.tile([1, 512], F32, tag="sum")
                    sq_ps = psE.tile([1, 512], F32, tag="sq")
                    for cb in range(NCB):
                        xr = pet.tile([128, 512], F32R, tag="xr")
                        nc.vector.tensor_copy(xr[:], x1[:, cb, sl])
                        x2 = pet.tile([128, 512], F32R, tag="x2")
                        nc.vector.tensor_tensor(x2[:], x1[:, cb, sl], x1[:, cb, sl], OP.mult)
                        nc.tensor.matmul(sum_ps[:], ones_r[:, 0:1], xr[:],
                                         start=(cb == 0), stop=(cb == NCB - 1))
                        nc.tensor.matmul(sq_ps[:], ones_r[:, 0:1], x2[:],
                                         start=(cb == 0), stop=(cb == NCB - 1))
                    nc.scalar.mul(m2v[:, sl], sum_ps[:], 1.0 / C)
                    nc.scalar.mul(v2[:, sl], sq_ps[:], 1.0 / C)
                ms2 = pd.tile([1, TOK], F32)
                nc.vector.tensor_mul(ms2[:], m2v[:], m2v[:])
                nc.vector.tensor_sub(v2[:], v2[:], ms2[:])
                nc.scalar.activation(v2[:], v2[:], AF.Sqrt, bias=eps_t[0:1, :])
                nc.vector.reciprocal(v2[:], v2[:])
                mean2_b = pd.tile([128, TOK], F32)
                rstd2_b = pd.tile([128, TOK], F32)
                nc.gpsimd.partition_broadcast(mean2_b[:], m2v[:])
                nc.gpsimd.partition_broadcast(rstd2_b[:], v2[:])
                for cb in range(NCB):
                    xc = pet.tile([128, TOK], F32, tag="xc2")
                    nc.vector.tensor_sub(xc[:], x1[:, cb, :], mean2_b[:])
                    nc.vector.tensor_tensor(ln2r[:, cb, :], xc[:], rstd2_b[:], OP.mult)

            # ============ Phase E: MLP ============
            with tc.tile_pool(name="ph", bufs=1) as ph, \
                 tc.tile_pool(name="pew", bufs=3) as pew, \
                 tc.tile_pool(name="pepw", bufs=2) as pepw, \
                 tc.tile_pool(name="peo", bufs=3) as peo, \
                 tc.tile_pool(name="psF", bufs=2, space="PSUM") as psF:
                for tcks in range(TOK // 512):
                    sl = slice(tcks * 512, (tcks + 1) * 512)
                    h_r = ph.tile([128, NFB, 512], F32R, tag="h")
                    for fb in range(NFB):
                        wt = pew.tile([128, NCB, 128], F32R, tag="fwt")
                        nc.sync.dma_start(wt[:], wfc_d[fb])
                        fc = psF.tile([128, 512], F32, tag="fc")
                        for cb in range(NCB):
                            nc.tensor.matmul(fc[:], wt[:, cb, :], ln2r[:, cb, sl],
                                             start=(cb == 0), stop=(cb == NCB - 1))
                        nc.scalar.activation(h_r[:, fb, :], fc[:], AF.Gelu)
                    for ocb in range(NCB):
                        acc = psF.tile([128, 512], F32, tag="pacc")
                        for fh in range(2):
                            wt = pepw.tile([128, NFB // 2, 128], F32R, tag="pwt")
                            nc.sync.dma_start(wt[:], wproj_d[ocb, fh])
                            for fi in range(NFB // 2):
                                fb = fh * (NFB // 2) + fi
                                nc.tensor.matmul(acc[:], wt[:, fi, :], h_r[:, fb, :],
                                                 start=(fb == 0), stop=(fb == NFB - 1))
                        of = peo.tile([128, 512], F32, tag="of")
                        nc.vector.tensor_add(of[:], acc[:], x1[:, ocb, sl])
                        nc.sync.dma_start(out_d[ocb * 128:(ocb + 1) * 128, sl], of[:])

    nc.compile()
    return nc


def _round_f32r(x):
    """fp32 -> float32r bit pattern: RNE to 11 explicit mantissa bits."""
    u = x.view(np.uint32).astype(np.uint64)
    bias = ((u >> 12) & 1) + (1 << 11) - 1
    u = (u + bias) >> 12 << 12
    return (u & 0xFFFFFFFF).astype(np.uint32).view(np.float32)


def _prep_weights(g1, w_qkv, w_o, g2, w_fc, w_proj):
    g1 = np.asarray(g1, np.float32)
    g2 = np.asarray(g2, np.float32)
    wqkvT = np.ascontiguousarray((np.asarray(w_qkv, np.float32) * g1[None, :]).T)
    woT = np.ascontiguousarray(np.asarray(w_o, np.float32).T)
    wfcT = np.ascontiguousarray((np.asarray(w_fc, np.float32) * g2[None, :]).T)
    wprojT = np.ascontiguousarray(np.asarray(w_proj, np.float32).T)

    # wqk[g, ocb, r, cb, f] = wqkvT[cb*128 + r, col0(g, ocb) + f]
    wqk = np.empty((NG, 2 * NPAIR, 128, NCB, 128), np.float32)
    wv = np.empty((NG, 128, NCB, 512), np.float32)
    for g in range(NG):
        for ocb in range(2 * NPAIR):
            col0 = (0 if ocb < NPAIR else C) + g * 512 + (ocb % NPAIR) * 128
            blk = wqkvT[:, col0:col0 + 128].reshape(NCB, 128, 128)  # [cb, r, f]
            wqk[g, ocb] = blk.transpose(1, 0, 2)
        vblk = wqkvT[:, 2 * C + g * 512:2 * C + (g + 1) * 512].reshape(NCB, 128, 512)
        wv[g] = vblk.transpose(1, 0, 2)
    wo = np.empty((NCB, 128, NCB, 128), np.float32)
    for ocb in range(NCB):
        blk = woT[:, ocb * 128:(ocb + 1) * 128].reshape(NCB, 128, 128)
        wo[ocb] = blk.transpose(1, 0, 2)
    wfc = np.empty((NFB, 128, NCB, 128), np.float32)
    for fb in range(NFB):
        blk = wfcT[:, fb * 128:(fb + 1) * 128].reshape(NCB, 128, 128)
        wfc[fb] = blk.transpose(1, 0, 2)
    wproj = np.empty((NCB, 2, 128, NFB // 2, 128), np.float32)
    for ocb in range(NCB):
        blk = wprojT[:, ocb * 128:(ocb + 1) * 128].reshape(NFB, 128, 128)  # [fb, r, f]
        for fh in range(2):
            wproj[ocb, fh] = blk[fh * (NFB // 2):(fh + 1) * (NFB // 2)].transpose(1, 0, 2)
    return {"wqk": _round_f32r(wqk), "wv": _round_f32r(wv), "wo": _round_f32r(wo),
            "wfc": _round_f32r(wfc), "wproj": _round_f32r(wproj)}


def _prep(x, g1, w_qkv, w_o, g2, w_fc, w_proj):
    """Build the 8 per-core input maps (all host-side)."""
    x = np.asarray(x, np.float32)
    wmap = _prep_weights(g1, w_qkv, w_o, g2, w_fc, w_proj)

    in_maps = []
    for c in range(8):
        b, h = c // 2, c % 2
        # Rotate the sequence so the core's own tokens are always chunk 1
        # ([TOK:T]) of xT: h=0 swaps halves, h=1 keeps order.
        xb = x[b]
        if h == 0:
            xb = np.concatenate([xb[TOK:], xb[:TOK]], axis=0)
        xT = np.ascontiguousarray(xb.T)
        # Causal masks in ROTATED key coordinates, packed [row, visit, 512].
        rot = (np.arange(T) + (TOK if h == 0 else 0)) % T
        gq = rot[TOK:]           # global positions of own (query) tokens
        gk = rot                 # global positions of keys in rotated order
        masks = np.zeros((NV, 128, 512), np.float32)
        for qc in range(2):
            trip = 12 + 4 * qc
            qpos = gq[qc * 512:(qc + 1) * 512]
            for i in range(4):
                j = trip - 4 + i
                kpos = gk[j * 128:(j + 1) * 128]
                masks[qc * 4 + i] = (kpos[:, None] <= qpos[None, :])
        masks = np.ascontiguousarray(masks.transpose(1, 0, 2))  # [128, NV, 512]
        # vmask: zero K/V rows never visible to any own query
        vmask = (gk <= gq.max()).astype(np.float32).reshape(NSB, 128).T
        vmask = np.ascontiguousarray(vmask)  # [128, NSB]
        in_maps.append({"xT": xT, "masks": masks.astype(ml_dtypes.bfloat16),
                        "vmask": vmask, **wmap})
    return in_maps


def kernel(x, g1, w_qkv, w_o, g2, w_fc, w_proj, _trace=False, **_tk):
    from concourse.bass_utils import run_bass_kernel_spmd
    if "nc" not in _CACHE:
        _CACHE["nc"] = _build()
    nc = _CACHE["nc"]
    in_maps = _prep(x, g1, w_qkv, w_o, g2, w_fc, w_proj)
    res = run_bass_kernel_spmd(nc, in_maps, core_ids=list(range(8)),
                               trace=_trace, **_tk)
    _CACHE["last"] = res
    out = np.empty((B, T, C), np.float32)
    for c in range(8):
        b, h = c // 2, c % 2
        out[b, h * TOK:(h + 1) * TOK, :] = res.results[c]["outT"].T
    return out
